# revision 32
# baseline (speedup 1.0000x reference)
"""AdaConv2D Trainium2 kernel.

Per sample (data-parallel over batch, one sample per NeuronCore):
  1. instance-norm x (mean/unbiased-std per channel, eps added to std)
  2. grouped 3x3 conv, 64 groups of 4->4 channels
  3. grouped 1x1 conv, 64 groups of 4->4 channels
  4. + per-channel bias

Key restructurings:
  * The 1x1 pointwise conv commutes with the 3x3 depthwise conv, so both
    merge into ONE grouped 3x3 conv with per-sample weights
    C[g,p,i,s] = sum_o PW[g,p,o] * DW[g,o,i,s]  (packed on host, block-diag).
  * The normalization folds into the conv: weights are scaled by 1/std of
    the input channel (on device; stats from on-device bn_stats over f32
    staging strips), the image border is padded with the per-channel MEAN
    (so that (pad - mean)/std == 0 matches the reference's zero-padding of
    the normalized image), and a per-output-channel offset
    offset[m] = bias[m] - sum_{k,s} W'[s][k,m] * mean[k]
    is applied during the PSUM->SBUF copy.
  * Conv runs as 9 shifted block-diagonal bf16 matmuls (N=512) accumulated
    in fp32 PSUM.
"""

import os
import sys

for _p in ("/opt/trn_rl_repo", "/opt/pypackages"):
    if _p not in sys.path:
        sys.path.insert(0, _p)

import numpy as np
from contextlib import ExitStack

import concourse.bass as bass
import concourse.tile as tile
from concourse import bacc, mybir
from concourse.bass_utils import run_bass_kernel_spmd

F32 = mybir.dt.float32
BF16 = mybir.dt.bfloat16
AF = mybir.ActivationFunctionType

B, C, H, W = 8, 256, 128, 128
G = 64                      # groups per sample
PW_ROW = W + 2              # padded row length (130)
PAD_LEN = (H + 2) * PW_ROW + 8
NCHUNK = H // 4             # 32 chunks of 4 output rows
VAR_SCALE = float(H * W) / float(H * W - 1)   # biased var -> ddof=1
EPS = 1e-7
N_CORES = 8
DIAG_TILES = False


def build_nc():
    nc = bacc.Bacc("TRN2", target_bir_lowering=False, debug=False)
    x_d = nc.dram_tensor("x", [C, H, PW_ROW], F32, kind="ExternalInput").ap()
    w_d = nc.dram_tensor("w", [2, 9, 128, 128], F32, kind="ExternalInput").ap()
    b_d = nc.dram_tensor("b", [2, 128], F32, kind="ExternalInput").ap()
    out_d = nc.dram_tensor("out", [C, H, W], F32, kind="ExternalOutput").ap()

    with ExitStack() as ctx:
        tc = ctx.enter_context(tile.TileContext(nc))
        xpool = ctx.enter_context(tc.tile_pool(name="xpool", bufs=2))
        fpool = ctx.enter_context(tc.tile_pool(name="fpool", bufs=1))
        wpool = ctx.enter_context(tc.tile_pool(name="wpool", bufs=3))
        spool = ctx.enter_context(tc.tile_pool(name="spool", bufs=2))
        opool = ctx.enter_context(tc.tile_pool(name="opool", bufs=6))
        ppool = ctx.enter_context(tc.tile_pool(name="ppool", bufs=8, space="PSUM"))

        def rows(xp, base, n):
            # [128, n, W] view of padded rows starting at linear offset base
            return xp[:, base : base + n * PW_ROW].rearrange(
                "p (r c) -> p r c", c=PW_ROW
            )[:, :, :W]

        warm = spool.tile([128, 1], F32, name="warm", tag="warm")
        nc.vector.memset(warm, 1.0)
        warm2 = spool.tile([128, 1], F32, name="warm2", tag="warm2")
        nc.scalar.activation(warm2, warm, AF.Sqrt)

        for h in range(2):
            ch0 = h * 128
            xp = xpool.tile([128, PAD_LEN], BF16, name=f"xp{h}", tag="xpad")
            # f32 padded staging (one slot shared by both halves)
            xf = fpool.tile([128, PAD_LEN], F32, name=f"xf{h}", tag="xf32")

            # host pads x cols with zeros; zero top/bottom bf16 pad rows
            nc.vector.memset(xp[:, 0:PW_ROW], 0.0)
            nc.vector.memset(xp[:, (H + 1) * PW_ROW : PAD_LEN], 0.0)
            inner = xp[:, PW_ROW : (H + 1) * PW_ROW].rearrange(
                "p (r c) -> p r c", c=PW_ROW
            )

            # DMA x strips into f32 padded staging; bn_stats per 4 rows
            # (one 512-elem group per instruction, raw emission — the HW op
            # streams the whole free AP as one group); contiguous cast
            # strip->bf16 on ScalarE
            stats = spool.tile([128, 32, 6], F32, name=f"st{h}", tag="stats")
            # strip DMAs issued up front (the issuing engines never run ops
            # that wait on these DMAs); the load is HBM-bandwidth-bound, the
            # fine 8-row granularity keeps the stats chase tail short
            for strip in range(16):
                r0 = strip * 8
                span0 = (r0 + 1) * PW_ROW
                eng = nc.sync if strip % 2 == 0 else nc.gpsimd
                eng.dma_start(
                    xf[:, span0 : span0 + 8 * PW_ROW],
                    x_d[ch0 : ch0 + 128, r0 : r0 + 8, :],
                )
            for strip in range(16):
                r0 = strip * 8
                base = (r0 + 1) * PW_ROW + 1
                span0 = (r0 + 1) * PW_ROW
                for q in range(2):
                    nc.vector.add_instruction(
                        mybir.InstBNStats(
                            name=nc.get_next_instruction_name(),
                            ins=[
                                nc.vector.lower_ap(
                                    rows(xf, base + q * 4 * PW_ROW, 4)
                                )
                            ],
                            outs=[
                                nc.vector.lower_ap(stats[:, strip * 2 + q, :])
                            ],
                        )
                    )
                if h == 0:
                    nc.scalar.activation(
                        xp[:, span0 : span0 + 8 * PW_ROW],
                        xf[:, span0 : span0 + 8 * PW_ROW],
                        AF.Copy,
                    )
                else:
                    # during h0's conv ScalarE must stay free for the
                    # PSUM-releasing activations; DVE is idle then
                    nc.vector.tensor_copy(
                        xp[:, span0 : span0 + 8 * PW_ROW],
                        xf[:, span0 : span0 + 8 * PW_ROW],
                    )

            aggr = spool.tile([128, 2], F32, name=f"ag{h}", tag="aggr")
            nc.vector.bn_aggr(aggr, stats)
            mean = aggr[:, 0:1]

            stdt = spool.tile([128, 1], F32, name=f"sd{h}", tag="stdt")
            nc.scalar.activation(stdt, aggr[:, 1:2], AF.Sqrt, scale=VAR_SCALE)
            stde = spool.tile([128, 1], F32, name=f"se{h}", tag="stde")
            nc.vector.tensor_scalar_add(stde, stdt, EPS)
            inv = spool.tile([128, 1], F32, name=f"iv{h}", tag="inv")
            nc.vector.reciprocal(inv, stde)

            # fill borders with mean: contiguous rows on ScalarE (parallel
            # to the DVE fold chain), strided columns on DVE (ScalarE pays
            # heavy per-run overhead on strided APs)
            top = xp[:, 0:PW_ROW]
            nc.scalar.activation(top, top, AF.Identity, bias=mean)
            bot = xp[:, (H + 1) * PW_ROW : (H + 2) * PW_ROW]
            nc.scalar.activation(bot, bot, AF.Identity, bias=mean)
            lcol = inner[:, :, 0:1]
            nc.vector.tensor_scalar_add(lcol, lcol, mean)
            rcol = inner[:, :, W + 1 : W + 2]
            nc.vector.tensor_scalar_add(rcol, rcol, mean)

            # weights: load f32, scale rows (input channels) by 1/std -> bf16
            ws_all = wpool.tile([128, 9 * 128], BF16, name=f"ws_{h}", tag="ws", bufs=2)
            ws = []
            for s in range(9):
                w0 = wpool.tile([128, 128], F32, name=f"w0_{h}_{s}", tag="w0", bufs=18)
                nc.sync.dma_start(w0, w_d[h, s])
                wt = ws_all[:, s * 128 : (s + 1) * 128]
                nc.vector.tensor_scalar_mul(wt, w0, inv)
                ws.append(wt)

            bias_sb = spool.tile([128, 1], F32, name=f"bs{h}", tag="bias")
            nc.sync.dma_start(bias_sb, b_d[h].unsqueeze(1))
            mean_b = spool.tile([128, 1], BF16, name=f"mb{h}", tag="meanb")
            nc.vector.tensor_copy(mean_b, mean)
            off = spool.tile([128, 1], F32, name=f"of{h}", tag="off")

            # main conv: 9 shifted block-diag matmuls accumulating in PSUM.
            # The bias/mean offset matvec rides on the PE after block 0 so it
            # doesn't delay the first conv matmul.
            BLK = 7
            for blk0 in range(0, NCHUNK, BLK):
                chunks = list(range(blk0, min(blk0 + BLK, NCHUNK)))
                psums = {}
                for cc in chunks:
                    psums[cc] = ppool.tile(
                        [128, 512], F32, name=f"ps{h}_{cc}", tag="ps"
                    )
                for s in range(9):
                    dy, dx = s // 3, s % 3
                    for cc in chunks:
                        rhs = rows(xp, (4 * cc + dy) * PW_ROW + dx, 4)
                        if DIAG_TILES:
                            # weights are block-diagonal: drive only the 4
                            # diagonal 32x32 PE sub-arrays (concurrent, same
                            # span, ~1/4 the array switching power)
                            for d in range(4):
                                p0 = 32 * d
                                nc.tensor.matmul(
                                    psums[cc][p0 : p0 + 32, :],
                                    ws[s][p0 : p0 + 32, p0 : p0 + 32],
                                    rhs[p0 : p0 + 32, :, :],
                                    start=(s == 0),
                                    stop=(s == 8),
                                    tile_position=(p0, p0),
                                )
                        else:
                            nc.tensor.matmul(
                                psums[cc], ws[s], rhs,
                                start=(s == 0), stop=(s == 8),
                            )
                if blk0 == 0:
                    # offset[m] = bias[m] - sum_s (ws_s^T @ mean)[m]
                    pso_t = ppool.tile([128, 512], F32, name=f"pso{h}", tag="ps")
                    pso = pso_t[:, 0:1]
                    for s in range(9):
                        nc.tensor.matmul(
                            pso, ws[s], mean_b, start=(s == 0), stop=(s == 8)
                        )
                    # off = bias - pso, on ScalarE: DVE's FIFO may be deep in
                    # the other half's stats when this becomes ready
                    nc.scalar.activation(
                        off, pso, AF.Identity, bias=bias_sb, scale=-1.0
                    )
                for cc in chunks:
                    ob = opool.tile([128, 512], F32, name=f"ob{h}_{cc}", tag="ob")
                    nc.scalar.activation(ob, psums[cc], AF.Identity, bias=off)
                    nc.sync.dma_start(
                        out_d[ch0 : ch0 + 128, 4 * cc : 4 * cc + 4, :],
                        ob.rearrange("p (r c) -> p r c", c=W),
                    )
    nc.finalize()
    return nc


def pack_weights(dw, pw):
    """Merge pointwise into depthwise and lay out as block-diagonal lhsT.

    dw: (256, 4, 3, 3), pw: (256, 4, 1, 1) for one sample.
    Returns (2, 9, 128, 128) f32: [half, tap, k=in-channel, m=out-channel].
    """
    DW = np.ascontiguousarray(dw, np.float32).reshape(G, 4, 4, 9)  # g,o,i,s
    PW = np.ascontiguousarray(pw, np.float32).reshape(G, 4, 4)     # g,p,o
    Cm = np.einsum("gpo,gois->gpis", PW, DW).astype(np.float32)    # g,p,i,s
    Wt = np.zeros((2, 9, 128, 128), np.float32)
    g = np.arange(G)
    hh = g // 32
    gl = g % 32
    for p4 in range(4):
        for i4 in range(4):
            # Wt[h, s, 4*gl+i4, 4*gl+p4] = Cm[g, p4, i4, s]
            Wt[hh, :, 4 * gl + i4, 4 * gl + p4] = Cm[:, p4, i4, :]
    return Wt


def make_in_maps(x, dw_kernels, pw_kernels, biases):
    xpad = np.zeros((B, C, H, PW_ROW), np.float32)
    xpad[:, :, :, 1 : W + 1] = x
    biases = np.ascontiguousarray(biases, np.float32).reshape(B, 2, 128)
    in_maps = []
    for i in range(B):
        in_maps.append(
            {
                "x": xpad[i],
                "w": pack_weights(dw_kernels[i], pw_kernels[i]),
                "b": biases[i],
            }
        )
    return in_maps


_NC_CACHE = []


def get_nc():
    if not _NC_CACHE:
        _NC_CACHE.append(build_nc())
    return _NC_CACHE[0]


def kernel(x, dw_kernels, pw_kernels, biases):
    nc = get_nc()
    in_maps = make_in_maps(x, dw_kernels, pw_kernels, biases)
    for attempt in range(3):
        res = run_bass_kernel_spmd(nc, in_maps, core_ids=list(range(N_CORES)))
        out = np.stack([r["out"] for r in res.results]).astype(np.float32)
        # rare transient device corruption has been observed (one NaN run in
        # ~25); the NEFF is stateless so a retry is safe
        if np.isfinite(out).all():
            return out
    return out


# revision 33
# speedup vs baseline: 1.0427x; 1.0427x over previous
"""AdaConv2D Trainium2 kernel.

Per sample (data-parallel over batch, one sample per NeuronCore):
  1. instance-norm x (mean/unbiased-std per channel, eps added to std)
  2. grouped 3x3 conv, 64 groups of 4->4 channels
  3. grouped 1x1 conv, 64 groups of 4->4 channels
  4. + per-channel bias

Key restructurings:
  * The 1x1 pointwise conv commutes with the 3x3 depthwise conv, so both
    merge into ONE grouped 3x3 conv with per-sample weights
    C[g,p,i,s] = sum_o PW[g,p,o] * DW[g,o,i,s]  (packed on host, block-diag).
  * The normalization folds into the conv: weights are scaled by 1/std of
    the input channel (on device; stats from on-device bn_stats over f32
    staging strips), the image border is padded with the per-channel MEAN
    (so that (pad - mean)/std == 0 matches the reference's zero-padding of
    the normalized image), and a per-output-channel offset
    offset[m] = bias[m] - sum_{k,s} W'[s][k,m] * mean[k]
    is applied during the PSUM->SBUF copy.
  * Conv runs as 9 shifted block-diagonal bf16 matmuls (N=512) accumulated
    in fp32 PSUM.
"""

import os
import sys

for _p in ("/opt/trn_rl_repo", "/opt/pypackages"):
    if _p not in sys.path:
        sys.path.insert(0, _p)

import numpy as np
from contextlib import ExitStack

import concourse.bass as bass
import concourse.tile as tile
from concourse import bacc, mybir
from concourse.bass_utils import run_bass_kernel_spmd

F32 = mybir.dt.float32
BF16 = mybir.dt.bfloat16
AF = mybir.ActivationFunctionType

B, C, H, W = 8, 256, 128, 128
G = 64                      # groups per sample
PW_ROW = W + 2              # padded row length (130)
PAD_LEN = (H + 2) * PW_ROW + 8
NCHUNK = H // 4             # 32 chunks of 4 output rows
VAR_SCALE = float(H * W) / float(H * W - 1)   # biased var -> ddof=1
EPS = 1e-7
N_CORES = 8
DIAG_TILES = False


def build_nc():
    nc = bacc.Bacc("TRN2", target_bir_lowering=False, debug=False)
    x_d = nc.dram_tensor("x", [C, H, PW_ROW], BF16, kind="ExternalInput").ap()
    w_d = nc.dram_tensor("w", [2, 9, 128, 128], F32, kind="ExternalInput").ap()
    b_d = nc.dram_tensor("b", [2, 128], F32, kind="ExternalInput").ap()
    out_d = nc.dram_tensor("out", [C, H, W], F32, kind="ExternalOutput").ap()

    with ExitStack() as ctx:
        tc = ctx.enter_context(tile.TileContext(nc))
        xpool = ctx.enter_context(tc.tile_pool(name="xpool", bufs=2))
        wpool = ctx.enter_context(tc.tile_pool(name="wpool", bufs=3))
        spool = ctx.enter_context(tc.tile_pool(name="spool", bufs=2))
        opool = ctx.enter_context(tc.tile_pool(name="opool", bufs=6))
        ppool = ctx.enter_context(tc.tile_pool(name="ppool", bufs=8, space="PSUM"))

        def rows(xp, base, n):
            # [128, n, W] view of padded rows starting at linear offset base
            return xp[:, base : base + n * PW_ROW].rearrange(
                "p (r c) -> p r c", c=PW_ROW
            )[:, :, :W]

        warm = spool.tile([128, 1], F32, name="warm", tag="warm")
        nc.vector.memset(warm, 1.0)
        warm2 = spool.tile([128, 1], F32, name="warm2", tag="warm2")
        nc.scalar.activation(warm2, warm, AF.Sqrt)

        for h in range(2):
            ch0 = h * 128
            xp = xpool.tile([128, PAD_LEN], BF16, name=f"xp{h}", tag="xpad")

            # host pads x cols with zeros; zero top/bottom bf16 pad rows
            nc.vector.memset(xp[:, 0:PW_ROW], 0.0)
            nc.vector.memset(xp[:, (H + 1) * PW_ROW : PAD_LEN], 0.0)
            inner = xp[:, PW_ROW : (H + 1) * PW_ROW].rearrange(
                "p (r c) -> p r c", c=PW_ROW
            )

            # DMA x strips into f32 padded staging; bn_stats per 4 rows
            # (one 512-elem group per instruction, raw emission — the HW op
            # streams the whole free AP as one group); contiguous cast
            # strip->bf16 on ScalarE
            stats = spool.tile([128, 32, 6], F32, name=f"st{h}", tag="stats")
            # strip DMAs issued up front (the issuing engines never run ops
            # that wait on these DMAs); the load is HBM-bandwidth-bound, the
            # fine 8-row granularity keeps the stats chase tail short
            for strip in range(16):
                r0 = strip * 8
                span0 = (r0 + 1) * PW_ROW
                eng = nc.sync if strip % 2 == 0 else nc.gpsimd
                eng.dma_start(
                    xp[:, span0 : span0 + 8 * PW_ROW],
                    x_d[ch0 : ch0 + 128, r0 : r0 + 8, :],
                )
            for strip in range(16):
                r0 = strip * 8
                base = (r0 + 1) * PW_ROW + 1
                for q in range(2):
                    nc.vector.add_instruction(
                        mybir.InstBNStats(
                            name=nc.get_next_instruction_name(),
                            ins=[
                                nc.vector.lower_ap(
                                    rows(xp, base + q * 4 * PW_ROW, 4)
                                )
                            ],
                            outs=[
                                nc.vector.lower_ap(stats[:, strip * 2 + q, :])
                            ],
                        )
                    )

            aggr = spool.tile([128, 2], F32, name=f"ag{h}", tag="aggr")
            nc.vector.bn_aggr(aggr, stats)
            mean = aggr[:, 0:1]

            stdt = spool.tile([128, 1], F32, name=f"sd{h}", tag="stdt")
            nc.scalar.activation(stdt, aggr[:, 1:2], AF.Sqrt, scale=VAR_SCALE)
            stde = spool.tile([128, 1], F32, name=f"se{h}", tag="stde")
            nc.vector.tensor_scalar_add(stde, stdt, EPS)
            inv = spool.tile([128, 1], F32, name=f"iv{h}", tag="inv")
            nc.vector.reciprocal(inv, stde)

            # fill borders with mean: contiguous rows on ScalarE (parallel
            # to the DVE fold chain), strided columns on DVE (ScalarE pays
            # heavy per-run overhead on strided APs)
            top = xp[:, 0:PW_ROW]
            nc.scalar.activation(top, top, AF.Identity, bias=mean)
            bot = xp[:, (H + 1) * PW_ROW : (H + 2) * PW_ROW]
            nc.scalar.activation(bot, bot, AF.Identity, bias=mean)
            lcol = inner[:, :, 0:1]
            nc.vector.tensor_scalar_add(lcol, lcol, mean)
            rcol = inner[:, :, W + 1 : W + 2]
            nc.vector.tensor_scalar_add(rcol, rcol, mean)

            # weights: load f32, scale rows (input channels) by 1/std -> bf16
            ws_all = wpool.tile([128, 9 * 128], BF16, name=f"ws_{h}", tag="ws", bufs=2)
            ws = []
            for s in range(9):
                w0 = wpool.tile([128, 128], F32, name=f"w0_{h}_{s}", tag="w0", bufs=18)
                nc.sync.dma_start(w0, w_d[h, s])
                wt = ws_all[:, s * 128 : (s + 1) * 128]
                nc.vector.tensor_scalar_mul(wt, w0, inv)
                ws.append(wt)

            bias_sb = spool.tile([128, 1], F32, name=f"bs{h}", tag="bias")
            nc.sync.dma_start(bias_sb, b_d[h].unsqueeze(1))
            mean_b = spool.tile([128, 1], BF16, name=f"mb{h}", tag="meanb")
            nc.vector.tensor_copy(mean_b, mean)
            off = spool.tile([128, 1], F32, name=f"of{h}", tag="off")

            # main conv: 9 shifted block-diag matmuls accumulating in PSUM.
            # The bias/mean offset matvec rides on the PE after block 0 so it
            # doesn't delay the first conv matmul.
            BLK = 7
            for blk0 in range(0, NCHUNK, BLK):
                chunks = list(range(blk0, min(blk0 + BLK, NCHUNK)))
                psums = {}
                for cc in chunks:
                    psums[cc] = ppool.tile(
                        [128, 512], F32, name=f"ps{h}_{cc}", tag="ps"
                    )
                for s in range(9):
                    dy, dx = s // 3, s % 3
                    for cc in chunks:
                        rhs = rows(xp, (4 * cc + dy) * PW_ROW + dx, 4)
                        if DIAG_TILES:
                            # weights are block-diagonal: drive only the 4
                            # diagonal 32x32 PE sub-arrays (concurrent, same
                            # span, ~1/4 the array switching power)
                            for d in range(4):
                                p0 = 32 * d
                                nc.tensor.matmul(
                                    psums[cc][p0 : p0 + 32, :],
                                    ws[s][p0 : p0 + 32, p0 : p0 + 32],
                                    rhs[p0 : p0 + 32, :, :],
                                    start=(s == 0),
                                    stop=(s == 8),
                                    tile_position=(p0, p0),
                                )
                        else:
                            nc.tensor.matmul(
                                psums[cc], ws[s], rhs,
                                start=(s == 0), stop=(s == 8),
                            )
                if blk0 == 0:
                    # offset[m] = bias[m] - sum_s (ws_s^T @ mean)[m]
                    pso_t = ppool.tile([128, 512], F32, name=f"pso{h}", tag="ps")
                    pso = pso_t[:, 0:1]
                    for s in range(9):
                        nc.tensor.matmul(
                            pso, ws[s], mean_b, start=(s == 0), stop=(s == 8)
                        )
                    # off = bias - pso, on ScalarE: DVE's FIFO may be deep in
                    # the other half's stats when this becomes ready
                    nc.scalar.activation(
                        off, pso, AF.Identity, bias=bias_sb, scale=-1.0
                    )
                for cc in chunks:
                    ob = opool.tile([128, 512], F32, name=f"ob{h}_{cc}", tag="ob")
                    nc.scalar.activation(ob, psums[cc], AF.Identity, bias=off)
                    nc.sync.dma_start(
                        out_d[ch0 : ch0 + 128, 4 * cc : 4 * cc + 4, :],
                        ob.rearrange("p (r c) -> p r c", c=W),
                    )
    nc.finalize()
    return nc


def pack_weights(dw, pw):
    """Merge pointwise into depthwise and lay out as block-diagonal lhsT.

    dw: (256, 4, 3, 3), pw: (256, 4, 1, 1) for one sample.
    Returns (2, 9, 128, 128) f32: [half, tap, k=in-channel, m=out-channel].
    """
    DW = np.ascontiguousarray(dw, np.float32).reshape(G, 4, 4, 9)  # g,o,i,s
    PW = np.ascontiguousarray(pw, np.float32).reshape(G, 4, 4)     # g,p,o
    Cm = np.einsum("gpo,gois->gpis", PW, DW).astype(np.float32)    # g,p,i,s
    Wt = np.zeros((2, 9, 128, 128), np.float32)
    g = np.arange(G)
    hh = g // 32
    gl = g % 32
    for p4 in range(4):
        for i4 in range(4):
            # Wt[h, s, 4*gl+i4, 4*gl+p4] = Cm[g, p4, i4, s]
            Wt[hh, :, 4 * gl + i4, 4 * gl + p4] = Cm[:, p4, i4, :]
    return Wt


def make_in_maps(x, dw_kernels, pw_kernels, biases):
    import ml_dtypes

    xpad = np.zeros((B, C, H, PW_ROW), ml_dtypes.bfloat16)
    xpad[:, :, :, 1 : W + 1] = np.asarray(x, np.float32).astype(ml_dtypes.bfloat16)
    biases = np.ascontiguousarray(biases, np.float32).reshape(B, 2, 128)
    in_maps = []
    for i in range(B):
        in_maps.append(
            {
                "x": xpad[i],
                "w": pack_weights(dw_kernels[i], pw_kernels[i]),
                "b": biases[i],
            }
        )
    return in_maps


_NC_CACHE = []


def get_nc():
    if not _NC_CACHE:
        _NC_CACHE.append(build_nc())
    return _NC_CACHE[0]


def kernel(x, dw_kernels, pw_kernels, biases):
    nc = get_nc()
    in_maps = make_in_maps(x, dw_kernels, pw_kernels, biases)
    for attempt in range(3):
        res = run_bass_kernel_spmd(nc, in_maps, core_ids=list(range(N_CORES)))
        out = np.stack([r["out"] for r in res.results]).astype(np.float32)
        # rare transient device corruption has been observed (one NaN run in
        # ~25); the NEFF is stateless so a retry is safe
        if np.isfinite(out).all():
            return out
    return out
